# revision 16
# baseline (speedup 1.0000x reference)
"""AttentionBlock kernel for 8 TRN2 NeuronCores.

Reference math (per sample s of 4, C=256 channels, HW=64*64=4096 positions):
  qkv = w_qkv @ x + b_qkv ; q,k,v = split(qkv)
  S = (q^T k) / sqrt(C) ; P = softmax(S, axis=-1)
  out = w_out @ (P @ v^T)^T + b_out + x

Sharding: core i -> (sample s=i//2, row half h=i%2, rows n0=h*2048 .. +2048).
K/V are computed for the full sample on both half-cores (duplicate compute is
cheap); Q and the attention rows only for the core's half.

On-chip layout: scores are computed transposed, S^T[m, n] (m = key position on
partitions, n = query row in free dim), so P^T = exp(S^T) is directly the
moving operand of the PV matmul (contraction over m = partitions) -- no
transposes anywhere. Softmax row sums come from an extra matmul with an
all-ones stationary operand (result is pre-broadcast across partitions);
normalization is folded into the PSUM->SBUF copy as a tensor*tensor multiply
with the reciprocal. Projection biases are folded into the matmuls by
augmenting x and the weights with a ones row (K=257); the 1/sqrt(C) scale is
folded into w_q on the host. All matmuls run in float32r (full PE rate).
"""

import sys

sys.path.insert(0, "/opt/trn_rl_repo")

import numpy as np

import concourse.bacc as bacc
import concourse.bass as bass
import concourse.tile as tile
from concourse import mybir

B = 4
C = 256
HW = 4096  # 64*64
NH = 2048  # rows per core (half sample)
NT = 512   # n-tile (free dim per psum bank)
F32 = mybir.dt.float32
F32R = mybir.dt.float32r

_CACHE = {}


def _body(nc, pools, variant="full"):
    const, xp, qk, vtp, ptp, ep, pss, pacc, pf, dram = pools
    xkv, xq, bias2, wtens, y, invs = dram

    # ---- load inputs ----
    xk0 = xp.tile([128, HW], F32R, tag="xk0", name="xk0")
    xk1 = xp.tile([128, HW], F32R, tag="xk1", name="xk1")
    xon = xp.tile([1, HW], F32R, tag="xon", name="xon")
    nxk = 1 if variant == "bigdma" else 4
    for q4 in range(nxk):
        w4 = HW // nxk
        qs = slice(q4 * w4, q4 * w4 + w4)
        nc.sync.dma_start(out=xk0[:, qs], in_=xkv.ap()[0:128, qs])
        nc.sync.dma_start(out=xk1[:, qs], in_=xkv.ap()[128:256, qs])
    nc.sync.dma_start(out=xon, in_=xkv.ap()[256:257, :])

    xq0 = xp.tile([128, NH], F32R, tag="xq0", name="xq0")
    xq1 = xp.tile([128, NH], F32R, tag="xq1", name="xq1")
    for q4 in range(max(1, nxk // 2)):
        w4 = NH // max(1, nxk // 2)
        qs = slice(q4 * w4, q4 * w4 + w4)
        nc.sync.dma_start(out=xq0[:, qs], in_=xq.ap()[0:128, qs])
        nc.sync.dma_start(out=xq1[:, qs], in_=xq.ap()[128:256, qs])
    bq = [None, None]
    bk = [None, None]
    bv = [None, None]
    for cb in range(2):
        bq[cb] = const.tile([128, 1], F32, tag=f"bq{cb}", name=f"bq{cb}")
        bk[cb] = const.tile([128, 1], F32, tag=f"bk{cb}", name=f"bk{cb}")
        bv[cb] = const.tile([128, 1], F32, tag=f"bv{cb}", name=f"bv{cb}")
        nc.sync.dma_start(out=bq[cb], in_=bias2.ap()[cb * 128:cb * 128 + 128, 0:1])
        nc.sync.dma_start(out=bk[cb], in_=bias2.ap()[cb * 128:cb * 128 + 128, 1:2])
        nc.sync.dma_start(out=bv[cb], in_=bias2.ap()[cb * 128:cb * 128 + 128, 2:3])

    ws = {}
    for name in ("wq", "wk", "wv", "wo"):
        t = wtens[name]
        w0 = const.tile([128, C], F32R, tag=name + "0", name=name + "0")
        w1 = const.tile([128, C], F32R, tag=name + "1", name=name + "1")
        nc.sync.dma_start(out=w0, in_=t.ap()[0:128, :])
        nc.sync.dma_start(out=w1, in_=t.ap()[128:256, :])
        if name in ("wo",):
            wb = const.tile([1, C], F32R, tag=name + "b", name=name + "b")
            nc.sync.dma_start(out=wb, in_=t.ap()[256:257, :])
        else:
            wb = None
        ws[name] = (w0, w1, wb)

    ones_f = const.tile([128, 1], F32, tag="ones_f", name="ones_f")
    nc.vector.memset(ones_f, 1.0)
    ones1 = const.tile([128, 1], F32R, tag="ones", name="ones")
    nc.vector.tensor_copy(ones1, ones_f)

    # ---- projections ----
    q_sb = [qk.tile([128, NH], F32R, tag=f"q{cb}", name=f"q{cb}") for cb in range(2)]
    k_sb = [qk.tile([128, HW], F32R, tag=f"k{cb}", name=f"k{cb}") for cb in range(2)]
    # vT[mb][128, C]: vT[m, c] = sum_ci x_aug[ci, m] * wv_aug[ci, c]
    vt_sb = [vtp.tile([128, C], F32R, tag=f"v{mb}", name=f"v{mb}")
             for mb in range(HW // 128)]

    w0, w1, wb = ws["wq"]
    for cb in range(2):
        for half in range(2):
            sl = slice(half * 1024, half * 1024 + 1024)
            ps = pss.tile([128, 1024], F32, tag="ps", name="ps_q")
            for j in range(2):
                s2 = slice(half * 1024 + j * 512, half * 1024 + j * 512 + 512)
                o2 = slice(j * 512, j * 512 + 512)
                cs = slice(cb * 128, cb * 128 + 128)
                nc.tensor.matmul(ps[:, o2], w0[:, cs], xq0[:, s2],
                                 start=True, stop=False)
                nc.tensor.matmul(ps[:, o2], w1[:, cs], xq1[:, s2],
                                 start=False, stop=True)
            nc.vector.tensor_scalar_add(q_sb[cb][:, sl], ps, bq[cb])

    w0, w1, wb = ws["wk"]
    for cb in range(2):
        for quad in range(4):
            sl = slice(quad * 1024, quad * 1024 + 1024)
            ps = pss.tile([128, 1024], F32, tag="ps", name="ps_k")
            for j in range(2):
                s2 = slice(quad * 1024 + j * 512, quad * 1024 + j * 512 + 512)
                o2 = slice(j * 512, j * 512 + 512)
                cs = slice(cb * 128, cb * 128 + 128)
                nc.tensor.matmul(ps[:, o2], w0[:, cs], xk0[:, s2],
                                 start=True, stop=False)
                nc.tensor.matmul(ps[:, o2], w1[:, cs], xk1[:, s2],
                                 start=False, stop=True)
            nc.scalar.activation(k_sb[cb][:, sl], ps,
                                 mybir.ActivationFunctionType.Identity,
                                 bias=bk[cb])

    w0, w1, wb = ws["wv"]
    for mb in range(HW // 128):
        ms = slice(mb * 128, mb * 128 + 128)
        pv = pacc.tile([128, NT], F32, tag=f"po{mb % 2}", name="pv")
        nc.tensor.matmul(pv[:, 0:C], xk0[:, ms], w0, start=True, stop=False)
        nc.tensor.matmul(pv[:, 0:C], xk1[:, ms], w1, start=False, stop=True)
        if mb % 2 == 0:
            nc.vector.tensor_copy(vt_sb[mb], pv[:, 0:C])
        else:
            nc.scalar.copy(vt_sb[mb], pv[:, 0:C])

    # ---- attention main loop ----
    wo0, wo1, wob = ws["wo"]
    n_mp = HW // 256  # m-pairs of 128 rows each
    for nt in range(NH // NT):
        nsl = slice(nt * NT, nt * NT + NT)
        po = [pacc.tile([128, NT], F32, tag=f"po{cb}", name=f"po{cb}")
              for cb in range(2)]
        psum = pacc.tile([1, NT], F32, tag="psum", name="psum")
        if variant == "noexp":
            nc.vector.memset(po[0], 0.0)
            nc.vector.memset(po[1], 0.0)
        for mp in range(n_mp):
            ps = pss.tile([128, 2 * NT], F32, tag="ps", name="ps_s")
            for j in range(2):  # two m-chunks per pair
                mb = 2 * mp + j
                msl = slice(mb * 128, mb * 128 + 128)
                osl = slice(j * NT, j * NT + NT)
                nc.tensor.matmul(ps[:, osl], k_sb[0][:, msl], q_sb[0][:, nsl],
                                 start=True, stop=False)
                nc.tensor.matmul(ps[:, osl], k_sb[1][:, msl], q_sb[1][:, nsl],
                                 start=False, stop=True)
            if variant == "noexp":
                continue
            pt = ptp.tile([128, 2 * NT], F32R, tag="pt", name="pt")
            nc.scalar.activation(pt, ps, mybir.ActivationFunctionType.Exp)
            first = mp == 0
            last = mp == n_mp - 1
            for j in range(2):
                mb = 2 * mp + j
                osl = slice(j * NT, j * NT + NT)
                st = first and j == 0
                sp = last and j == 1
                nc.tensor.matmul(po[0], vt_sb[mb][:, 0:128], pt[:, osl],
                                 start=st, stop=sp, skip_group_check=True)
                nc.tensor.matmul(po[1], vt_sb[mb][:, 128:256], pt[:, osl],
                                 start=st, stop=sp, skip_group_check=True)
                if variant != "nosum":
                    nc.tensor.matmul(psum, ones1, pt[:, osl],
                                     start=st, stop=sp, skip_group_check=True)

        # epilogue for this n-tile
        inv = ep.tile([128, NT], F32, tag="inv", name="inv")
        if variant in ("nosum", "noexp"):
            nc.vector.memset(inv, 1.0)
        else:
            inv1 = ep.tile([1, NT], F32, tag="inv1", name="inv1")
            scr = ep.tile([1, NT], F32, tag="scr", name="scr")
            nc.vector.reciprocal_approx_accurate(inv1, psum, scr)
            nc.sync.dma_start(out=invs.ap()[nt:nt + 1, :], in_=inv1)
            src_row = invs.ap()[nt, :]
            nc.sync.dma_start(out=inv, in_=bass.AP(
                tensor=src_row.tensor, offset=src_row.offset,
                ap=[[0, 128]] + list(src_row.ap)))
        ou = [ep.tile([128, NT], F32R, tag=f"ou{cb}", name=f"ou{cb}")
              for cb in range(2)]
        nc.vector.tensor_mul(ou[0], po[0], inv)
        nc.vector.tensor_mul(ou[1], po[1], inv)
        nc.vector.tensor_scalar_add(ou[0], ou[0], bv[0])
        nc.vector.tensor_scalar_add(ou[1], ou[1], bv[1])
        for ob in range(2):
            cs = slice(ob * 128, ob * 128 + 128)
            pff = pf.tile([128, NT], F32, tag="pf", name="pff")
            nc.tensor.matmul(pff, wo0[:, cs], ou[0], start=True, stop=False)
            nc.tensor.matmul(pff, wo1[:, cs], ou[1], start=False, stop=False)
            nc.tensor.matmul(pff, wob[0:1, cs], xon[0:1, 0:NT],
                             start=False, stop=True)
            fin = ep.tile([128, NT], F32, tag=f"fin{ob}", name=f"fin{ob}")
            xres = xq0 if ob == 0 else xq1
            nc.vector.tensor_add(fin, pff, xres[:, nsl].bitcast(F32))
            nc.sync.dma_start(out=y.ap()[cs, nsl], in_=fin)


def _emit(nc, reps=0, variant="full"):
    xkv = nc.dram_tensor("xkv", (C + 1, HW), F32R, kind="ExternalInput")
    xq = nc.dram_tensor("xq", (C, NH), F32R, kind="ExternalInput")
    bias2 = nc.dram_tensor("bias2", (C, 3), F32, kind="ExternalInput")
    wtens = {n: nc.dram_tensor(n, (C + 1, C), F32R, kind="ExternalInput")
             for n in ("wq", "wk", "wv", "wo")}
    y = nc.dram_tensor("y", (C, NH), F32, kind="ExternalOutput")
    invs = nc.dram_tensor("invs", (NH // NT, NT), F32)
    dram = (xkv, xq, bias2, wtens, y, invs)

    with tile.TileContext(nc) as tc:
        with (
            tc.tile_pool(name="const", bufs=1) as const,
            tc.tile_pool(name="xp", bufs=1) as xp,
            tc.tile_pool(name="qk", bufs=1) as qk,
            tc.tile_pool(name="vt", bufs=1) as vtp,
            tc.tile_pool(name="pt", bufs=4) as ptp,
            tc.tile_pool(name="ep", bufs=2) as ep,
            tc.tile_pool(name="pss", bufs=2, space="PSUM") as pss,
            tc.tile_pool(name="pacc", bufs=1, space="PSUM") as pacc,
            tc.tile_pool(name="pf", bufs=1, space="PSUM") as pf,
        ):
            pools = (const, xp, qk, vtp, ptp, ep, pss, pacc, pf, dram)
            if reps:
                with tc.For_i(0, reps, 1, hint_engines=(
                        mybir.EngineType.PE, mybir.EngineType.Activation,
                        mybir.EngineType.DVE)):
                    _body(nc, pools, variant)
            else:
                _body(nc, pools, variant)
    return nc


def _build(reps=0, variant="full"):
    key = ("nc", reps, variant)
    if key not in _CACHE:
        nc = bacc.Bacc("TRN2", target_bir_lowering=False, debug=False,
                       num_devices=8)
        _emit(nc, reps=reps, variant=variant)
        nc.compile()
        _CACHE[key] = nc
    return _CACHE[key]


def make_in_maps(x, w_qkv, b_qkv, w_out, b_out):
    scale = 1.0 / np.sqrt(C)
    waug = {
        "wq": np.ascontiguousarray(
            np.vstack([w_qkv[0:C].T, np.zeros((1, C))]) * scale,
            dtype=np.float32),
        "wk": np.ascontiguousarray(
            np.vstack([w_qkv[C:2 * C].T, np.zeros((1, C))]),
            dtype=np.float32),
        "wv": np.ascontiguousarray(
            np.vstack([w_qkv[2 * C:3 * C].T, np.zeros((1, C))]),
            dtype=np.float32),
        "wo": np.ascontiguousarray(
            np.vstack([w_out.T, b_out[None]]), dtype=np.float32),
        "bias2": np.ascontiguousarray(
            np.stack([b_qkv[0:C] * scale, b_qkv[C:2 * C],
                      b_qkv[2 * C:3 * C]], axis=1),
            dtype=np.float32),
    }
    x4 = x.reshape(B, C, HW)
    in_maps = []
    for i in range(8):
        s, h = i // 2, i % 2
        xkv = np.empty((C + 1, HW), dtype=np.float32)
        xkv[0:C] = x4[s]
        xkv[C] = 1.0
        m = {"xkv": xkv,
             "xq": np.ascontiguousarray(x4[s][:, h * NH:(h + 1) * NH])}
        m.update(waug)
        in_maps.append(m)
    return in_maps


def _get_runner():
    """Build the 8-core PJRT executable once; reuse across kernel() calls."""
    if "runner" in _CACHE:
        return _CACHE["runner"]
    import jax
    from jax.experimental.shard_map import shard_map
    from jax.sharding import Mesh, PartitionSpec
    from concourse import mybir as _mybir
    from concourse.bass2jax import _bass_exec_p, install_neuronx_cc_hook

    nc = _build()
    install_neuronx_cc_hook()
    n_cores = 8
    devices = jax.devices()[:n_cores]
    mesh = Mesh(np.asarray(devices), ("core",))

    in_names, out_names, out_avals, zero_outs = [], [], [], []
    for alloc in nc.m.functions[0].allocations:
        if not isinstance(alloc, _mybir.MemoryLocationSet):
            continue
        name = alloc.memorylocations[0].name
        if alloc.kind == "ExternalInput":
            in_names.append(name)
        elif alloc.kind == "ExternalOutput":
            out_names.append(name)
            shape = tuple(alloc.tensor_shape)
            dtype = _mybir.dt.np(alloc.dtype)
            out_avals.append(jax.core.ShapedArray(shape, dtype))
            zero_outs.append(np.zeros((n_cores * shape[0], *shape[1:]), dtype))

    def _bodyf(*args):
        return tuple(_bass_exec_p.bind(
            *args, out_avals=tuple(out_avals),
            in_names=tuple(in_names + out_names), out_names=tuple(out_names),
            lowering_input_output_aliases=(), sim_require_finite=True,
            sim_require_nnan=True, nc=nc))

    nin = len(in_names) + len(out_names)
    fn = jax.jit(shard_map(_bodyf, mesh=mesh,
                           in_specs=(PartitionSpec("core"),) * nin,
                           out_specs=(PartitionSpec("core"),) * len(out_names),
                           check_rep=False), keep_unused=True)
    pid_name = nc.partition_id_tensor.name if nc.partition_id_tensor else None

    def run(in_maps):
        args = []
        for nm in in_names:
            if nm == pid_name:
                args.append(np.arange(n_cores, dtype=np.uint32).reshape(n_cores, 1))
            else:
                args.append(np.concatenate(
                    [np.asarray(in_maps[c][nm]) for c in range(n_cores)], 0))
        args += zero_outs
        outs = fn(*args)
        per_core = []
        for c in range(n_cores):
            per_core.append({
                nm: np.asarray(outs[i]).reshape(n_cores, *out_avals[i].shape)[c]
                for i, nm in enumerate(out_names)})
        return per_core

    _CACHE["runner"] = run
    return run


def kernel(x, w_qkv, b_qkv, w_out, b_out):
    x = np.asarray(x, dtype=np.float32)
    w_qkv = np.asarray(w_qkv, dtype=np.float32)
    b_qkv = np.asarray(b_qkv, dtype=np.float32)
    w_out = np.asarray(w_out, dtype=np.float32)
    b_out = np.asarray(b_out, dtype=np.float32)

    in_maps = make_in_maps(x, w_qkv, b_qkv, w_out, b_out)
    results = _get_runner()(in_maps)

    out = np.empty((B, C, HW), dtype=np.float32)
    for i in range(8):
        s, h = i // 2, i % 2
        out[s, :, h * NH:(h + 1) * NH] = results[i]["y"]
    return out.reshape(B, C, 64, 64)


# revision 24
# speedup vs baseline: 1.3406x; 1.3406x over previous
"""AttentionBlock kernel for 8 TRN2 NeuronCores.

Reference math (per sample s of 4, C=256 channels, HW=64*64=4096 positions):
  qkv = w_qkv @ x + b_qkv ; q,k,v = split(qkv)
  S = (q^T k) / sqrt(C) ; P = softmax(S, axis=-1)
  out = w_out @ (P @ v^T)^T + b_out + x

Sharding: core i -> (sample s=i//2, row half h=i%2, rows n0=h*2048 .. +2048).
K/V are computed for the full sample on both half-cores (duplicate compute is
cheap); Q and the attention rows only for the core's half.

On-chip layout: scores are computed transposed, S^T[m, n] (m = key position on
partitions, n = query row in free dim), so P^T = exp(S^T) is directly the
moving operand of the PV matmul (contraction over m = partitions) -- no
transposes anywhere. Softmax row sums come from an extra matmul with an
all-ones stationary operand (result is pre-broadcast across partitions);
normalization is folded into the PSUM->SBUF copy as a tensor*tensor multiply
with the reciprocal (computed on one partition, broadcast via a DRAM-bounce
DMA). q/k biases ride the PSUM->SBUF copies as per-partition scalars; the v
bias is added to the normalized attention output (commutes through PV); the
out-proj bias uses a ones-row augmentation of w_out. The 1/sqrt(C) scale is
folded into w_q on the host. Projections and the output projection run in
float32r (1 cyc/col); the attention main loop (scores, PV, softmax sums) runs
in fp8e4m3 with perf_mode=DoubleRow (K=256 per matmul at 0.5 cyc/col, ~2.2x
faster than f32r). exp applies a constant -3.25 shift (cancels in P/sum) so
the largest probability (~119) stays below the fp8e4 max-finite ~240 (this
float8e4 HAS an inf encoding; overflow -> inf -> NaN). Scale-relative absmax
error ~4e-3 (fp8 quantization noise averaged through the softmax).

The reps/variant build parameters exist only for the timing harness
(test.py/bench2.py): reps wraps the body in an on-device For_i loop, variants
disable phases for attribution.
"""

import sys

sys.path.insert(0, "/opt/trn_rl_repo")

import numpy as np

import concourse.bacc as bacc
import concourse.bass as bass
import concourse.tile as tile
from concourse import mybir

B = 4
C = 256
HW = 4096  # 64*64
NH = 2048  # rows per core (half sample)
NT = 512   # n-tile (free dim per psum bank)
F32 = mybir.dt.float32
F32R = mybir.dt.float32r
FP8 = mybir.dt.float8e4

_CACHE = {}


def _body(nc, pools, variant="full"):
    const, xp, qk, vtp, ptp, ep, pss, pacc, pf, dram = pools
    xkv, xq, bias2, wtens, y, invs, dbg = dram

    # ---- load inputs ----
    xk0 = xp.tile([128, HW], F32R, tag="xk0", name="xk0")
    xk1 = xp.tile([128, HW], F32R, tag="xk1", name="xk1")
    xon = xp.tile([1, HW], F32R, tag="xon", name="xon")
    nxk = 1 if variant == "bigdma" else 4
    for q4 in range(nxk):
        w4 = HW // nxk
        qs = slice(q4 * w4, q4 * w4 + w4)
        nc.sync.dma_start(out=xk0[:, qs], in_=xkv.ap()[0:128, qs])
        nc.sync.dma_start(out=xk1[:, qs], in_=xkv.ap()[128:256, qs])
    nc.sync.dma_start(out=xon, in_=xkv.ap()[256:257, :])

    xq0 = xp.tile([128, NH], F32R, tag="xq0", name="xq0")
    xq1 = xp.tile([128, NH], F32R, tag="xq1", name="xq1")
    for q4 in range(max(1, nxk // 2)):
        w4 = NH // max(1, nxk // 2)
        qs = slice(q4 * w4, q4 * w4 + w4)
        nc.sync.dma_start(out=xq0[:, qs], in_=xq.ap()[0:128, qs])
        nc.sync.dma_start(out=xq1[:, qs], in_=xq.ap()[128:256, qs])
    bq = [None, None]
    bk = [None, None]
    bv = [None, None]
    for cb in range(2):
        bq[cb] = const.tile([128, 1], F32, tag=f"bq{cb}", name=f"bq{cb}")
        bk[cb] = const.tile([128, 1], F32, tag=f"bk{cb}", name=f"bk{cb}")
        bv[cb] = const.tile([128, 1], F32, tag=f"bv{cb}", name=f"bv{cb}")
        nc.sync.dma_start(out=bq[cb], in_=bias2.ap()[cb * 128:cb * 128 + 128, 0:1])
        nc.sync.dma_start(out=bk[cb], in_=bias2.ap()[cb * 128:cb * 128 + 128, 1:2])
        nc.sync.dma_start(out=bv[cb], in_=bias2.ap()[cb * 128:cb * 128 + 128, 2:3])

    ws = {}
    for name in ("wq", "wk", "wv", "wo"):
        t = wtens[name]
        w0 = const.tile([128, C], F32R, tag=name + "0", name=name + "0")
        w1 = const.tile([128, C], F32R, tag=name + "1", name=name + "1")
        nc.sync.dma_start(out=w0, in_=t.ap()[0:128, :])
        nc.sync.dma_start(out=w1, in_=t.ap()[128:256, :])
        if name in ("wo",):
            wb = const.tile([1, C], F32R, tag=name + "b", name=name + "b")
            nc.sync.dma_start(out=wb, in_=t.ap()[256:257, :])
        else:
            wb = None
        ws[name] = (w0, w1, wb)

    ones_f = const.tile([128, 2, 16], F32, tag="ones_f", name="ones_f")
    nc.vector.memset(ones_f, 1.0)
    ones2 = const.tile([128, 2, 16], FP8, tag="ones", name="ones")
    nc.vector.tensor_copy(ones2, ones_f)
    eshift = const.tile([128, 1], F32, tag="eshift", name="eshift")
    nc.vector.memset(eshift, -3.25)

    # ---- projections ----
    q8 = qk.tile([128, 2, NH], FP8, tag="q8", name="q8")
    k8 = qk.tile([128, 2, HW], FP8, tag="k8", name="k8")
    # vT pair tiles [m 128, khalf 2, c 256] for DoubleRow PV
    vt8 = [vtp.tile([128, 2, C], FP8, tag=f"v{mp}", name=f"v{mp}")
           for mp in range(HW // 256)]

    w0, w1, wb = ws["wq"]
    for cb in range(2):
        for half in range(2):
            sl = slice(half * 1024, half * 1024 + 1024)
            ps = pss.tile([128, 1024], F32, tag="ps", name="ps_q")
            for j in range(2):
                s2 = slice(half * 1024 + j * 512, half * 1024 + j * 512 + 512)
                o2 = slice(j * 512, j * 512 + 512)
                cs = slice(cb * 128, cb * 128 + 128)
                nc.tensor.matmul(ps[:, o2], w0[:, cs], xq0[:, s2],
                                 start=True, stop=False)
                nc.tensor.matmul(ps[:, o2], w1[:, cs], xq1[:, s2],
                                 start=False, stop=True)
            nc.vector.tensor_scalar_add(q8[:, cb, sl], ps, bq[cb])

    w0, w1, wb = ws["wk"]
    for cb in range(2):
        for quad in range(4):
            sl = slice(quad * 1024, quad * 1024 + 1024)
            ps = pss.tile([128, 1024], F32, tag="ps", name="ps_k")
            for j in range(2):
                s2 = slice(quad * 1024 + j * 512, quad * 1024 + j * 512 + 512)
                o2 = slice(j * 512, j * 512 + 512)
                cs = slice(cb * 128, cb * 128 + 128)
                nc.tensor.matmul(ps[:, o2], w0[:, cs], xk0[:, s2],
                                 start=True, stop=False)
                nc.tensor.matmul(ps[:, o2], w1[:, cs], xk1[:, s2],
                                 start=False, stop=True)
            nc.scalar.activation(k8[:, cb, sl], ps,
                                 mybir.ActivationFunctionType.Identity,
                                 bias=bk[cb])

    w0, w1, wb = ws["wv"]
    for mb in range(HW // 128):
        ms = slice(mb * 128, mb * 128 + 128)
        pv = pacc.tile([128, NT], F32, tag=f"po{mb % 2}", name="pv")
        nc.tensor.matmul(pv[:, 0:C], xk0[:, ms], w0, start=True, stop=False)
        nc.tensor.matmul(pv[:, 0:C], xk1[:, ms], w1, start=False, stop=True)
        if mb % 2 == 0:
            nc.vector.tensor_copy(vt8[mb // 2][:, 0, :], pv[:, 0:C])
        else:
            nc.scalar.copy(vt8[mb // 2][:, 1, :], pv[:, 0:C])

    # ---- attention main loop ----
    wo0, wo1, wob = ws["wo"]
    n_mp = HW // 256  # m-pairs of 128 rows each
    for nt in range(NH // NT):
        nsl = slice(nt * NT, nt * NT + NT)
        po = [pacc.tile([128, NT], F32, tag=f"po{cb}", name=f"po{cb}")
              for cb in range(2)]
        psum = pacc.tile([1, NT], F32, tag="psum", name="psum")
        if variant == "noexp":
            nc.vector.memset(po[0], 0.0)
            nc.vector.memset(po[1], 0.0)
        for mp in range(n_mp):
            ps = pss.tile([128, 2 * NT], F32, tag="ps", name="ps_s")
            for j in range(2):  # two m-chunks per pair, one DoubleRow mm each
                mb = 2 * mp + j
                msl = slice(mb * 128, mb * 128 + 128)
                osl = slice(j * NT, j * NT + NT)
                nc.tensor.matmul(ps[:, osl], k8[:, :, msl], q8[:, :, nsl],
                                 start=True, stop=True,
                                 perf_mode=mybir.MatmulPerfMode.DoubleRow)
            if variant == "noexp":
                continue
            pt = ptp.tile([128, 2, NT], FP8, tag="pt", name="pt")
            ptf = pt.rearrange("p a b -> p (a b)")
            # shift by -2.5 (cancels in P/sum) so exp stays within fp8e4m3
            # range (max ~448) even for tail scores
            nc.scalar.activation(ptf, ps, mybir.ActivationFunctionType.Exp,
                                 scale=float(1.0 / np.sqrt(C)), bias=eshift)
            if dbg is not None and nt == 0 and mp == 0:
                nc.sync.dma_start(out=dbg["dq"].ap(), in_=q8)
                nc.sync.dma_start(out=dbg["dk"].ap(), in_=k8)
                nc.sync.dma_start(out=dbg["dv"].ap(), in_=vt8[0])
                psf = ep.tile([128, 2 * NT], F32, tag="psf", name="psf")
                nc.vector.tensor_copy(psf, ps)
                nc.sync.dma_start(out=dbg["dps"].ap(), in_=psf)
                nc.sync.dma_start(out=dbg["dpt"].ap(), in_=pt)
            st = mp == 0
            sp = mp == n_mp - 1
            nc.tensor.matmul(po[0], vt8[mp][:, :, 0:128], pt,
                             start=st, stop=sp, skip_group_check=True,
                             perf_mode=mybir.MatmulPerfMode.DoubleRow)
            nc.tensor.matmul(po[1], vt8[mp][:, :, 128:256], pt,
                             start=st, stop=sp, skip_group_check=True,
                             perf_mode=mybir.MatmulPerfMode.DoubleRow)
            if variant != "nosum":
                nc.tensor.matmul(psum, ones2[:, :, 0:1], pt,
                                 start=st, stop=sp, skip_group_check=True,
                                 perf_mode=mybir.MatmulPerfMode.DoubleRow)

        if dbg is not None and nt == 3:
            pof = ep.tile([128, NT], F32, tag="pof", name="pof")
            nc.vector.tensor_copy(pof, po[0])
            nc.sync.dma_start(out=dbg["dpo"].ap(), in_=pof)
            psmf = ep.tile([1, NT], F32, tag="psmf", name="psmf")
            nc.vector.tensor_copy(psmf, psum)
            nc.sync.dma_start(out=dbg["dsum"].ap(), in_=psmf)
        # epilogue for this n-tile
        inv = ep.tile([128, NT], F32, tag="inv", name="inv")
        if variant in ("nosum", "noexp"):
            nc.vector.memset(inv, 1.0)
        else:
            inv1 = ep.tile([1, NT], F32, tag="inv1", name="inv1")
            scr = ep.tile([1, NT], F32, tag="scr", name="scr")
            nc.vector.reciprocal_approx_accurate(inv1, psum, scr)
            nc.sync.dma_start(out=invs.ap()[nt:nt + 1, :], in_=inv1)
            src_row = invs.ap()[nt, :]
            nc.sync.dma_start(out=inv, in_=bass.AP(
                tensor=src_row.tensor, offset=src_row.offset,
                ap=[[0, 128]] + list(src_row.ap)))
        ou = [ep.tile([128, NT], F32R, tag=f"ou{cb}", name=f"ou{cb}")
              for cb in range(2)]
        nc.vector.tensor_mul(ou[0], po[0], inv)
        nc.vector.tensor_mul(ou[1], po[1], inv)
        nc.vector.tensor_scalar_add(ou[0], ou[0], bv[0])
        nc.vector.tensor_scalar_add(ou[1], ou[1], bv[1])
        for ob in range(2):
            cs = slice(ob * 128, ob * 128 + 128)
            pff = pf.tile([128, NT], F32, tag="pf", name="pff")
            nc.tensor.matmul(pff, wo0[:, cs], ou[0], start=True, stop=False)
            nc.tensor.matmul(pff, wo1[:, cs], ou[1], start=False, stop=False)
            nc.tensor.matmul(pff, wob[0:1, cs], xon[0:1, 0:NT],
                             start=False, stop=True)
            fin = ep.tile([128, NT], F32, tag=f"fin{ob}", name=f"fin{ob}")
            xres = xq0 if ob == 0 else xq1
            nc.vector.tensor_add(fin, pff, xres[:, nsl].bitcast(F32))
            nc.sync.dma_start(out=y.ap()[cs, nsl], in_=fin)


def _emit(nc, reps=0, variant="full"):
    xkv = nc.dram_tensor("xkv", (C + 1, HW), F32R, kind="ExternalInput")
    xq = nc.dram_tensor("xq", (C, NH), F32R, kind="ExternalInput")
    bias2 = nc.dram_tensor("bias2", (C, 3), F32, kind="ExternalInput")
    wtens = {n: nc.dram_tensor(n, (C + 1, C), F32R, kind="ExternalInput")
             for n in ("wq", "wk", "wv", "wo")}
    y = nc.dram_tensor("y", (C, NH), F32, kind="ExternalOutput")
    invs = nc.dram_tensor("invs", (NH // NT, NT), F32)
    dbg = None
    if variant == "dbg":
        dbg = {
            "dq": nc.dram_tensor("dq", (128, 2, NH), mybir.dt.float8e4,
                                 kind="ExternalOutput"),
            "dk": nc.dram_tensor("dk", (128, 2, HW), mybir.dt.float8e4,
                                 kind="ExternalOutput"),
            "dv": nc.dram_tensor("dv", (128, 2, C), mybir.dt.float8e4,
                                 kind="ExternalOutput"),
            "dps": nc.dram_tensor("dps", (128, 2 * NT), F32,
                                  kind="ExternalOutput"),
            "dpt": nc.dram_tensor("dpt", (128, 2, NT), mybir.dt.float8e4,
                                  kind="ExternalOutput"),
            "dpo": nc.dram_tensor("dpo", (128, NT), F32, kind="ExternalOutput"),
            "dsum": nc.dram_tensor("dsum", (1, NT), F32, kind="ExternalOutput"),
        }
    dram = (xkv, xq, bias2, wtens, y, invs, dbg)

    with tile.TileContext(nc) as tc:
        with (
            tc.tile_pool(name="const", bufs=1) as const,
            tc.tile_pool(name="xp", bufs=1) as xp,
            tc.tile_pool(name="qk", bufs=1) as qk,
            tc.tile_pool(name="vt", bufs=1) as vtp,
            tc.tile_pool(name="pt", bufs=4) as ptp,
            tc.tile_pool(name="ep", bufs=2) as ep,
            tc.tile_pool(name="pss", bufs=2, space="PSUM") as pss,
            tc.tile_pool(name="pacc", bufs=1, space="PSUM") as pacc,
            tc.tile_pool(name="pf", bufs=1, space="PSUM") as pf,
        ):
            pools = (const, xp, qk, vtp, ptp, ep, pss, pacc, pf, dram)
            if reps:
                with tc.For_i(0, reps, 1, hint_engines=(
                        mybir.EngineType.PE, mybir.EngineType.Activation,
                        mybir.EngineType.DVE)):
                    _body(nc, pools, variant)
            else:
                _body(nc, pools, variant)
    return nc


def _build(reps=0, variant="full"):
    key = ("nc", reps, variant)
    if key not in _CACHE:
        nc = bacc.Bacc("TRN2", target_bir_lowering=False, debug=False,
                       num_devices=8)
        _emit(nc, reps=reps, variant=variant)
        nc.compile()
        _CACHE[key] = nc
    return _CACHE[key]


def make_in_maps(x, w_qkv, b_qkv, w_out, b_out):
    waug = {
        "wq": np.ascontiguousarray(
            np.vstack([w_qkv[0:C].T, np.zeros((1, C))]),
            dtype=np.float32),
        "wk": np.ascontiguousarray(
            np.vstack([w_qkv[C:2 * C].T, np.zeros((1, C))]),
            dtype=np.float32),
        "wv": np.ascontiguousarray(
            np.vstack([w_qkv[2 * C:3 * C].T, np.zeros((1, C))]),
            dtype=np.float32),
        "wo": np.ascontiguousarray(
            np.vstack([w_out.T, b_out[None]]), dtype=np.float32),
        "bias2": np.ascontiguousarray(
            np.stack([b_qkv[0:C], b_qkv[C:2 * C],
                      b_qkv[2 * C:3 * C]], axis=1),
            dtype=np.float32),
    }
    x4 = x.reshape(B, C, HW)
    in_maps = []
    for i in range(8):
        s, h = i // 2, i % 2
        xkv = np.empty((C + 1, HW), dtype=np.float32)
        xkv[0:C] = x4[s]
        xkv[C] = 1.0
        m = {"xkv": xkv,
             "xq": np.ascontiguousarray(x4[s][:, h * NH:(h + 1) * NH])}
        m.update(waug)
        in_maps.append(m)
    return in_maps


def _get_runner():
    """Build the 8-core PJRT executable once; reuse across kernel() calls."""
    if "runner" in _CACHE:
        return _CACHE["runner"]
    import jax
    from jax.experimental.shard_map import shard_map
    from jax.sharding import Mesh, PartitionSpec
    from concourse import mybir as _mybir
    from concourse.bass2jax import _bass_exec_p, install_neuronx_cc_hook

    nc = _build()
    install_neuronx_cc_hook()
    n_cores = 8
    devices = jax.devices()[:n_cores]
    mesh = Mesh(np.asarray(devices), ("core",))

    in_names, out_names, out_avals, zero_outs = [], [], [], []
    for alloc in nc.m.functions[0].allocations:
        if not isinstance(alloc, _mybir.MemoryLocationSet):
            continue
        name = alloc.memorylocations[0].name
        if alloc.kind == "ExternalInput":
            in_names.append(name)
        elif alloc.kind == "ExternalOutput":
            out_names.append(name)
            shape = tuple(alloc.tensor_shape)
            dtype = _mybir.dt.np(alloc.dtype)
            out_avals.append(jax.core.ShapedArray(shape, dtype))
            zero_outs.append(np.zeros((n_cores * shape[0], *shape[1:]), dtype))

    def _bodyf(*args):
        return tuple(_bass_exec_p.bind(
            *args, out_avals=tuple(out_avals),
            in_names=tuple(in_names + out_names), out_names=tuple(out_names),
            lowering_input_output_aliases=(), sim_require_finite=True,
            sim_require_nnan=True, nc=nc))

    nin = len(in_names) + len(out_names)
    fn = jax.jit(shard_map(_bodyf, mesh=mesh,
                           in_specs=(PartitionSpec("core"),) * nin,
                           out_specs=(PartitionSpec("core"),) * len(out_names),
                           check_rep=False), keep_unused=True)
    pid_name = nc.partition_id_tensor.name if nc.partition_id_tensor else None

    def run(in_maps):
        args = []
        for nm in in_names:
            if nm == pid_name:
                args.append(np.arange(n_cores, dtype=np.uint32).reshape(n_cores, 1))
            else:
                args.append(np.concatenate(
                    [np.asarray(in_maps[c][nm]) for c in range(n_cores)], 0))
        args += zero_outs
        outs = fn(*args)
        per_core = []
        for c in range(n_cores):
            per_core.append({
                nm: np.asarray(outs[i]).reshape(n_cores, *out_avals[i].shape)[c]
                for i, nm in enumerate(out_names)})
        return per_core

    _CACHE["runner"] = run
    return run


def kernel(x, w_qkv, b_qkv, w_out, b_out):
    x = np.asarray(x, dtype=np.float32)
    w_qkv = np.asarray(w_qkv, dtype=np.float32)
    b_qkv = np.asarray(b_qkv, dtype=np.float32)
    w_out = np.asarray(w_out, dtype=np.float32)
    b_out = np.asarray(b_out, dtype=np.float32)

    in_maps = make_in_maps(x, w_qkv, b_qkv, w_out, b_out)
    results = _get_runner()(in_maps)

    out = np.empty((B, C, HW), dtype=np.float32)
    for i in range(8):
        s, h = i // 2, i % 2
        out[s, :, h * NH:(h + 1) * NH] = results[i]["y"]
    return out.reshape(B, C, 64, 64)
